# revision 15
# baseline (speedup 1.0000x reference)
"""Trainium2 Bass kernel for nn_BiAttentionLayer (T=8192, D=128), 8 NeuronCores.

Math: with context c, question q, kernel w = [w_c | w_q | w_m]:
    S[i,j] = c_i.w_c + q_j.w_q + (c_i*w_m).q_j
    A = softmax_rows(S);  U_A = A @ q
    b = rowmax(A);  h = b @ c;  G = [c, U_A, c*U_A, c*H_A]

Sharding: context rows are split across 8 cores (sequence-parallel over the
rows of the T x T score matrix); q is replicated. Softmax shift is a single
analytic per-core constant K_r (safe fp32 range), removing the row-max pass:
    Pt[j,i] = exp(q_j.(c_i*w_m) + qw_j - K_r)      (j on partitions)

Per core, 64 j-chunks of 128 are processed in pairs (pt2 tiles hold 2
chunks side by side):
    PE : S.T chunk = qT_chunk.T @ cmT            (fp16, fp32 PSUM)
    ACT: pt2 half = exp(S.T + bias)              (bf16 out)
    PE : U.T += qn_chunk.T @ pt half             (2x512-col MMs per chunk)
    PE : Z quads: ones.T @ pt 512-col slices, col-tiled to partitions
         0/32/64/96 of one PSUM bank (concurrent col-group matmuls)
    DVE: one running elementwise max per pair on the full pt2 tile

U and the running max are split into first/second-half accumulators so half
the output DMAs overlap compute. Final row-direction reductions (sum/max
over partitions, b = m/Z, U_A = U.T.T/Z, h, G assembly) run on host.

The output G is (8192, 512) float32.
"""

import sys
from contextlib import ExitStack

import numpy as np

for _p in ("/opt/trn_rl_repo",):
    if _p not in sys.path:
        sys.path.insert(0, _p)

T = 8192
D = 128
NCORES = 8
TS = T // NCORES  # 1024 context rows per core

_CACHE = {}


def _build_nc():
    import concourse.bass as bass
    import concourse.mybir as mybir
    import concourse.tile as tile
    from concourse import bacc

    F32 = mybir.dt.float32
    BF16 = mybir.dt.bfloat16
    F16 = mybir.dt.float16

    NJ = T // 128   # 64 j-chunks
    NP = NJ // 2    # 32 chunk pairs
    NN = TS // 512  # 2 psum column chunks
    PIPE = 1        # consume pair lag
    # Schraudolph exp-on-DVE: u16 = round(A16*s + 128*(127-sigma)) saturates
    # negatives to 0 and its bits ARE bf16(exp(s)) to ~1.8% rms. Offloading
    # some chunks' exp to the Vector engine takes them off the ACT pacer.
    A16 = 128.0 / float(np.log(2.0))
    B16 = 16249.0

    nc = bacc.Bacc("TRN2", target_bir_lowering=False, debug=False)

    qT_d = nc.declare_dram_parameter("qT", [128, T], F16, isOutput=False)
    cmT_d = nc.declare_dram_parameter("cmT", [128, TS], F16, isOutput=False)
    qn_d = nc.declare_dram_parameter("qn", [128, T], BF16, isOutput=False)
    ones_d = nc.declare_dram_parameter("ones", [128, 16], BF16, isOutput=False)
    qwb_d = nc.declare_dram_parameter("qwb", [128, NJ], F32, isOutput=False)

    uta_d = nc.declare_dram_parameter("uta", [128, TS], F32, isOutput=True)
    utb_d = nc.declare_dram_parameter("utb", [128, TS], F32, isOutput=True)
    z4_d = nc.declare_dram_parameter("z4", [4, 512], F32, isOutput=True)
    ma_d = nc.declare_dram_parameter("ma", [128, TS], BF16, isOutput=True)
    mb_d = nc.declare_dram_parameter("mb", [128, TS], BF16, isOutput=True)

    with tile.TileContext(nc) as tc, ExitStack() as ctx:
        const_pool = ctx.enter_context(tc.tile_pool(name="const", bufs=1))
        st_pool = ctx.enter_context(
            tc.tile_pool(name="st", bufs=2, space=bass.MemorySpace.PSUM)
        )
        acc_pool = ctx.enter_context(
            tc.tile_pool(name="acc", bufs=1, space=bass.MemorySpace.PSUM)
        )
        pt_pool = ctx.enter_context(tc.tile_pool(name="pt", bufs=4))

        u_ps = [
            acc_pool.tile([128, 512], F32, tag=f"u{n}", name=f"u{n}")
            for n in range(NN)
        ]
        z4_ps = acc_pool.tile([128, 512], F32, tag="z4", name="z4")

        # ACT table preload: a dummy 1-col exp issued before any data
        # arrives so the ~2.7us ACT_TABLE_LOAD overlaps the input DMAs.
        dummy = const_pool.tile([128, 1], F32, tag="dummy")
        nc.vector.memset(dummy[:], 0.0)
        nc.scalar.activation(
            dummy[:], dummy[:], mybir.ActivationFunctionType.Exp
        )

        # PE warmup spin: full-array matmuls with no DMA deps so the HAM
        # clock-gate reaches K=8/8 while the input DMAs stream in. Results
        # go to u_ps[0], which chunk 0's start=True accumulation clears.
        wm = const_pool.tile([128, 512], BF16, tag="wm")
        nc.vector.memset(wm[:], 0.5)
        for _w in range(4):
            nc.tensor.matmul(
                u_ps[0][:], wm[:, 0:128], wm[:], start=True, stop=True,
                skip_group_check=True,
            )

        NCHUNK = NJ // 8
        cmt_sb = const_pool.tile([128, TS], F16, tag="cmt")
        qt_tiles = [
            const_pool.tile([128, NCHUNK * 128], F16, tag=f"qt{k}", name=f"qt{k}")
            for k in range(8)
        ]
        qn_sb = const_pool.tile([128, NJ * 128], BF16, tag="qn")
        qwb_sb = const_pool.tile([128, NJ], F32, tag="qwb")
        qwb2_sb = const_pool.tile([128, NJ], F32, tag="qwb2")
        act_scr = const_pool.tile([128, 1], F32, tag="act_scr")
        ones_sb = const_pool.tile([128, 16], BF16, tag="ones")
        macc = [
            const_pool.tile([128, 2 * TS], BF16, tag=f"macc{h}", name=f"macc{h}")
            for h in range(2)
        ]
        nc.vector.memset(macc[0][:], 0.0)
        nc.vector.memset(macc[1][:], 0.0)
        u_sb = [
            const_pool.tile([128, TS], F32, tag=f"usb{h}", name=f"usb{h}")
            for h in range(2)
        ]
        m_sb = [
            const_pool.tile([128, TS], BF16, tag=f"msb{h}", name=f"msb{h}")
            for h in range(2)
        ]
        z_sb = const_pool.tile([128, 512], F32, tag="zsb")

        # DMAs, critical-first; per-chunk loads interleaved
        nc.sync.dma_start(cmt_sb[:], cmT_d.ap())
        sl0 = slice(0, NCHUNK * 128)
        nc.sync.dma_start(qt_tiles[0][:], qT_d.ap()[:, sl0])
        nc.sync.dma_start(qwb_sb[:], qwb_d.ap())
        nc.sync.dma_start(ones_sb[:], ones_d.ap())
        # ACT-side touch absorbing the qwb DMA wait
        nc.scalar.copy(act_scr[:], qwb_sb[:, 0:1])
        # per-chunk bias for the DVE-Schraudolph chunks: A16*qwb + B16
        nc.vector.tensor_scalar(
            qwb2_sb[:], qwb_sb[:], A16, B16,
            mybir.AluOpType.mult, mybir.AluOpType.add,
        )
        nc.sync.dma_start(qn_sb[:, sl0], qn_d.ap()[:, sl0])
        for k in range(1, 8):
            sl = slice(k * NCHUNK * 128, (k + 1) * NCHUNK * 128)
            nc.sync.dma_start(qt_tiles[k][:], qT_d.ap()[:, sl])
            nc.sync.dma_start(qn_sb[:, sl], qn_d.ap()[:, sl])

        # PE "touch" matmuls: absorb each DMA's completion wait on the PE so
        # real matmuls carry at most one semaphore wait. Results land in a
        # corner of the current S.T psum tile (overwritten by start=True).
        def pe_touch(ap, st):
            w = min(16, ap.shape[1])
            nc.tensor.matmul(
                st[0:1, 0:w], ap[:, 0:1], ap[:, 0:w],
                start=True, stop=True, skip_group_check=True,
            )

        HALF_P = NP // 2  # u/m half boundary (pairs 0..15 -> half 0)

        def emit_consume(p, pt2):
            half = 0 if p < HALF_P else 1
            # U.T accumulation: 4 x 512-col MMs (2 per chunk)
            for cc in range(2):  # chunk within pair
                jj = 2 * p + cc
                qslice = qn_sb[:, jj * 128 : (jj + 1) * 128]
                for n in range(NN):
                    sl = slice(cc * TS + n * 512, cc * TS + (n + 1) * 512)
                    nc.tensor.matmul(
                        u_ps[n][:], qslice, pt2[:, sl],
                        start=(p % HALF_P) == 0 and cc == 0,
                        stop=(p % HALF_P) == HALF_P - 1 and cc == 1,
                    )
            # Z quads: 4 col-tiled single-row MMs accumulating into
            # partitions 0/32/64/96 of one PSUM bank (concurrent col groups)
            for qd in range(4):
                c = 32 * qd
                nc.tensor.matmul(
                    z4_ps[c : c + 1, :],
                    ones_sb[:, 0:1],
                    pt2[:, qd * 512 : (qd + 1) * 512],
                    start=p == 0, stop=p == NP - 1,
                    skip_group_check=True,
                    tile_position=(0, c),
                )
            # running elementwise max over the whole pair tile
            nc.vector.tensor_max(macc[half][:], macc[half][:], pt2[:])
            if p == HALF_P - 1 or p == NP - 1:
                # drain this half: U psum -> SBUF -> DRAM, fold + ship max.
                # Copies split across Scalar+Vector; the mid-kernel max fold
                # goes to the otherwise-idle GpSimd.
                nc.scalar.copy(u_sb[half][:, 0:512], u_ps[0][:])
                nc.vector.tensor_copy(u_sb[half][:, 512:1024], u_ps[1][:])
                nc.sync.dma_start(
                    (uta_d if half == 0 else utb_d).ap(), u_sb[half][:]
                )
                nc.vector.tensor_max(
                    m_sb[half][:], macc[half][:, 0:TS], macc[half][:, TS:]
                )
                nc.sync.dma_start(
                    (ma_d if half == 0 else mb_d).ap(), m_sb[half][:]
                )

        pending = []
        for p in range(NP):
            # consume the oldest pending pair FIRST so the Vector FIFO runs
            # MAX(p-1) ahead of this pair's Schraudolph tensor_scalar
            if len(pending) > PIPE - 1:
                emit_consume(*pending.pop(0))
            pt2 = pt_pool.tile([128, 2 * TS], BF16)
            for cc in range(2):
                jj = 2 * p + cc
                st = st_pool.tile([128, TS], F32)
                if jj == 0:
                    pe_touch(ones_sb[:], st)
                    pe_touch(cmt_sb[:], st)
                if jj % NCHUNK == 0 and jj // NCHUNK < 2:
                    # touches only while input DMAs are still in flight
                    k = jj // NCHUNK
                    pe_touch(qt_tiles[k][:], st)
                    pe_touch(qn_sb[:, jj * 128 : jj * 128 + 16], st)
                qk = qt_tiles[jj // NCHUNK]
                off = (jj % NCHUNK) * 128
                for n in range(NN):
                    sl = slice(n * 512, (n + 1) * 512)
                    nc.tensor.matmul(
                        st[:, sl], qk[:, off : off + 128], cmt_sb[:, sl],
                        start=True, stop=True,
                    )
                if p % 2 == 1 and cc == 1:
                    # Schraudolph exp on the Vector engine (u16 bits = bf16)
                    nc.vector.tensor_scalar(
                        pt2[:, cc * TS : (cc + 1) * TS].bitcast(
                            mybir.dt.uint16
                        ),
                        st[:], A16, qwb2_sb[:, jj : jj + 1],
                        mybir.AluOpType.mult, mybir.AluOpType.add,
                    )
                else:
                    nc.scalar.activation(
                        pt2[:, cc * TS : (cc + 1) * TS], st[:],
                        mybir.ActivationFunctionType.Exp,
                        bias=qwb_sb[:, jj : jj + 1],
                    )
            pending.append((p, pt2))
        while pending:
            emit_consume(*pending.pop(0))

        # Z output: copy the z4 bank to SBUF (ScalarE reads PSUM fast),
        # then one partition-strided 8KB DMA of rows 0/32/64/96.
        nc.scalar.copy(z_sb[:], z4_ps[:])
        nc.sync.dma_start(z4_d.ap(), z_sb[0:97:32, :])

    nc.compile()
    return nc


def _host_inputs(c, q, qw, cm):
    import ml_dtypes

    NJ = T // 128
    qT = np.ascontiguousarray(q.T).astype(np.float16)
    qn_re = np.ascontiguousarray(
        q.reshape(NJ, 128, 128).transpose(1, 0, 2).reshape(128, T)
    ).astype(ml_dtypes.bfloat16)
    ones = np.ones((128, 16), dtype=ml_dtypes.bfloat16)
    in_maps = []
    for r in range(NCORES):
        rows = slice(r * TS, (r + 1) * TS)
        cm_r = cm[rows]
        sig2 = (cm_r.astype(np.float64) ** 2).sum(1)
        K = float(qw.max()) + 3.5 * float(np.sqrt(sig2.max()))
        in_maps.append(
            {
                "qT": qT,
                "cmT": np.ascontiguousarray(cm_r.T).astype(np.float16),
                "qn": qn_re,
                "ones": ones,
                "qwb": np.ascontiguousarray(
                    (qw - K).reshape(NJ, 128).T
                ).astype(np.float32),
            }
        )
    return in_maps


def kernel(x, kernel):
    from concourse.bass_utils import run_bass_kernel_spmd

    x = np.asarray(x, dtype=np.float32)
    kern = np.asarray(kernel, dtype=np.float32)
    c, q = x[0, 0], x[1, 0]
    w_c, w_q, w_m = kern[:D], kern[D : 2 * D], kern[2 * D :]

    qw = (q.astype(np.float64) @ w_q.astype(np.float64)).astype(np.float32)
    cm = (c * w_m[None, :]).astype(np.float32)

    if "nc" not in _CACHE:
        _CACHE["nc"] = _build_nc()
    nc = _CACHE["nc"]

    in_maps = _host_inputs(c, q, qw, cm)
    res = run_bass_kernel_spmd(nc, in_maps, list(range(NCORES)))

    U = np.empty((T, D), dtype=np.float64)
    Z = np.empty(T, dtype=np.float64)
    M = np.empty(T, dtype=np.float64)
    for r in range(NCORES):
        rows = slice(r * TS, (r + 1) * TS)
        out = res.results[r]
        U[rows] = (
            np.asarray(out["uta"], dtype=np.float64)
            + np.asarray(out["utb"], dtype=np.float64)
        ).T
        z4 = np.asarray(out["z4"], dtype=np.float64)  # (4, 512)
        zr = np.empty(TS, dtype=np.float64)
        zr[:512] = z4[0] + z4[2]
        zr[512:] = z4[1] + z4[3]
        Z[rows] = zr
        M[rows] = np.maximum(
            np.asarray(out["ma"], dtype=np.float64),
            np.asarray(out["mb"], dtype=np.float64),
        ).max(0)

    U_A = U / Z[:, None]
    b = M / Z
    h = b @ c.astype(np.float64)
    c64 = c.astype(np.float64)
    G = np.concatenate([c64, U_A, c64 * U_A, c64 * h[None, :]], axis=1)
    return G.astype(np.float32)


# revision 16
# speedup vs baseline: 1.1560x; 1.1560x over previous
"""Trainium2 Bass kernel for nn_BiAttentionLayer (T=8192, D=128), 8 NeuronCores.

Math: with context c, question q, kernel w = [w_c | w_q | w_m]:
    S[i,j] = c_i.w_c + q_j.w_q + (c_i*w_m).q_j
    A = softmax_rows(S);  U_A = A @ q
    b = rowmax(A);  h = b @ c;  G = [c, U_A, c*U_A, c*H_A]

Sharding: context rows are split across 8 cores (sequence-parallel over the
rows of the T x T score matrix); q is replicated. Softmax shift is a single
analytic per-core constant K_r (safe fp32 range), removing the row-max pass:
    Pt[j,i] = exp(q_j.(c_i*w_m) + qw_j - K_r)      (j on partitions)

Per core, 64 j-chunks of 128 are processed in pairs (pt2 tiles hold 2
chunks side by side):
    PE : S.T chunk = qT_chunk.T @ cmT            (fp16, fp32 PSUM)
    ACT: pt2 half = exp(S.T + bias)              (bf16 out)
    PE : U.T += qn_chunk.T @ pt half             (2x512-col MMs per chunk)
    PE : Z quads: ones.T @ pt 512-col slices, col-tiled to partitions
         0/32/64/96 of one PSUM bank (concurrent col-group matmuls)
    DVE: one running elementwise max per pair on the full pt2 tile

U and the running max are split into first/second-half accumulators so half
the output DMAs overlap compute. Final row-direction reductions (sum/max
over partitions, b = m/Z, U_A = U.T.T/Z, h, G assembly) run on host.

The output G is (8192, 512) float32.
"""

import sys
from contextlib import ExitStack

import numpy as np

for _p in ("/opt/trn_rl_repo",):
    if _p not in sys.path:
        sys.path.insert(0, _p)

T = 8192
D = 128
NCORES = 8
TS = T // NCORES  # 1024 context rows per core

_CACHE = {}


def _build_nc():
    import concourse.bass as bass
    import concourse.mybir as mybir
    import concourse.tile as tile
    from concourse import bacc

    F32 = mybir.dt.float32
    BF16 = mybir.dt.bfloat16
    F16 = mybir.dt.float16

    NJ = T // 128   # 64 j-chunks
    NP = NJ // 2    # 32 chunk pairs
    NN = TS // 512  # 2 psum column chunks
    PIPE = 1        # consume pair lag
    # Schraudolph exp-on-DVE: u16 = round(A16*s + 128*(127-sigma)) saturates
    # negatives to 0 and its bits ARE bf16(exp(s)) to ~1.8% rms. Offloading
    # some chunks' exp to the Vector engine takes them off the ACT pacer.
    A16 = 128.0 / float(np.log(2.0))
    B16 = 16249.0

    nc = bacc.Bacc("TRN2", target_bir_lowering=False, debug=False)

    qT_d = nc.declare_dram_parameter("qT", [128, T], F16, isOutput=False)
    cmT_d = nc.declare_dram_parameter("cmT", [128, TS], F16, isOutput=False)
    qn_d = nc.declare_dram_parameter("qn", [128, T], BF16, isOutput=False)
    ones_d = nc.declare_dram_parameter("ones", [128, 16], BF16, isOutput=False)
    qwb_d = nc.declare_dram_parameter("qwb", [128, NJ], F32, isOutput=False)

    uta_d = nc.declare_dram_parameter("uta", [128, TS], F32, isOutput=True)
    utb_d = nc.declare_dram_parameter("utb", [128, TS], F32, isOutput=True)
    z4_d = nc.declare_dram_parameter("z4", [4, 512], F32, isOutput=True)
    ma_d = nc.declare_dram_parameter("ma", [128, TS], BF16, isOutput=True)
    mb_d = nc.declare_dram_parameter("mb", [128, TS], BF16, isOutput=True)

    with tile.TileContext(nc) as tc, ExitStack() as ctx:
        const_pool = ctx.enter_context(tc.tile_pool(name="const", bufs=1))
        st_pool = ctx.enter_context(
            tc.tile_pool(name="st", bufs=2, space=bass.MemorySpace.PSUM)
        )
        acc_pool = ctx.enter_context(
            tc.tile_pool(name="acc", bufs=1, space=bass.MemorySpace.PSUM)
        )
        pt_pool = ctx.enter_context(tc.tile_pool(name="pt", bufs=4))

        u_ps = [
            acc_pool.tile([128, 512], F32, tag=f"u{n}", name=f"u{n}")
            for n in range(NN)
        ]
        z4_ps = acc_pool.tile([128, 512], F32, tag="z4", name="z4")

        # ACT table preload: a dummy 1-col exp issued before any data
        # arrives so the ~2.7us ACT_TABLE_LOAD overlaps the input DMAs.
        dummy = const_pool.tile([128, 1], F32, tag="dummy")
        nc.vector.memset(dummy[:], 0.0)
        nc.scalar.activation(
            dummy[:], dummy[:], mybir.ActivationFunctionType.Exp
        )

        # PE warmup spin: full-array matmuls with no DMA deps so the HAM
        # clock-gate reaches K=8/8 while the input DMAs stream in. Results
        # go to u_ps[0], which chunk 0's start=True accumulation clears.
        wm = const_pool.tile([128, 512], BF16, tag="wm")
        nc.vector.memset(wm[:], 0.5)
        for _w in range(4):
            nc.tensor.matmul(
                u_ps[0][:], wm[:, 0:128], wm[:], start=True, stop=True,
                skip_group_check=True,
            )

        NCHUNK = NJ // 8
        cmt_sb = const_pool.tile([128, TS], F16, tag="cmt")
        qt_tiles = [
            const_pool.tile([128, NCHUNK * 128], F16, tag=f"qt{k}", name=f"qt{k}")
            for k in range(8)
        ]
        qn_sb = const_pool.tile([128, NJ * 128], BF16, tag="qn")
        qwb_sb = const_pool.tile([128, NJ], F32, tag="qwb")
        qwb2_sb = const_pool.tile([128, NJ], F32, tag="qwb2")
        act_scr = const_pool.tile([128, 1], F32, tag="act_scr")
        ones_sb = const_pool.tile([128, 16], BF16, tag="ones")
        macc = [
            const_pool.tile([128, 2 * TS], BF16, tag=f"macc{h}", name=f"macc{h}")
            for h in range(2)
        ]
        nc.vector.memset(macc[0][:], 0.0)
        nc.vector.memset(macc[1][:], 0.0)
        u_sb = [
            const_pool.tile([128, TS], F32, tag=f"usb{h}", name=f"usb{h}")
            for h in range(2)
        ]
        m_sb = [
            const_pool.tile([128, TS], BF16, tag=f"msb{h}", name=f"msb{h}")
            for h in range(2)
        ]
        z_sb = const_pool.tile([128, 512], F32, tag="zsb")

        # DMAs, critical-first; per-chunk loads interleaved
        nc.sync.dma_start(cmt_sb[:], cmT_d.ap())
        sl0 = slice(0, NCHUNK * 128)
        nc.sync.dma_start(qt_tiles[0][:], qT_d.ap()[:, sl0])
        nc.sync.dma_start(qwb_sb[:], qwb_d.ap())
        nc.sync.dma_start(ones_sb[:], ones_d.ap())
        # ACT-side touch absorbing the qwb DMA wait
        nc.scalar.copy(act_scr[:], qwb_sb[:, 0:1])
        # per-chunk bias for the DVE-Schraudolph chunks: A16*qwb + B16
        nc.vector.tensor_scalar(
            qwb2_sb[:], qwb_sb[:], A16, B16,
            mybir.AluOpType.mult, mybir.AluOpType.add,
        )
        nc.sync.dma_start(qn_sb[:, sl0], qn_d.ap()[:, sl0])
        for k in range(1, 8):
            sl = slice(k * NCHUNK * 128, (k + 1) * NCHUNK * 128)
            nc.sync.dma_start(qt_tiles[k][:], qT_d.ap()[:, sl])
            nc.sync.dma_start(qn_sb[:, sl], qn_d.ap()[:, sl])

        # PE "touch" matmuls: absorb each DMA's completion wait on the PE so
        # real matmuls carry at most one semaphore wait. Results land in a
        # corner of the current S.T psum tile (overwritten by start=True).
        def pe_touch(ap, st):
            w = min(16, ap.shape[1])
            nc.tensor.matmul(
                st[0:1, 0:w], ap[:, 0:1], ap[:, 0:w],
                start=True, stop=True, skip_group_check=True,
            )

        HALF_P = NP // 2  # u/m half boundary (pairs 0..15 -> half 0)

        def emit_consume(p, pt2):
            half = 0 if p < HALF_P else 1
            # U.T accumulation: 4 x 512-col MMs (2 per chunk)
            for cc in range(2):  # chunk within pair
                jj = 2 * p + cc
                qslice = qn_sb[:, jj * 128 : (jj + 1) * 128]
                for n in range(NN):
                    sl = slice(cc * TS + n * 512, cc * TS + (n + 1) * 512)
                    nc.tensor.matmul(
                        u_ps[n][:], qslice, pt2[:, sl],
                        start=(p % HALF_P) == 0 and cc == 0,
                        stop=(p % HALF_P) == HALF_P - 1 and cc == 1,
                    )
            # Z quads: 4 col-tiled single-row MMs accumulating into
            # partitions 0/32/64/96 of one PSUM bank (concurrent col groups)
            for qd in range(4):
                c = 32 * qd
                nc.tensor.matmul(
                    z4_ps[c : c + 1, :],
                    ones_sb[:, 0:1],
                    pt2[:, qd * 512 : (qd + 1) * 512],
                    start=p == 0, stop=p == NP - 1,
                    skip_group_check=True,
                    tile_position=(0, c),
                )
            # running elementwise max over the whole pair tile
            nc.vector.tensor_max(macc[half][:], macc[half][:], pt2[:])
            if p == HALF_P - 1 or p == NP - 1:
                # drain this half: U psum -> SBUF -> DRAM, fold + ship max.
                # Copies split across Scalar+Vector; the mid-kernel max fold
                # goes to the otherwise-idle GpSimd.
                nc.scalar.copy(u_sb[half][:, 0:512], u_ps[0][:])
                nc.vector.tensor_copy(u_sb[half][:, 512:1024], u_ps[1][:])
                nc.sync.dma_start(
                    (uta_d if half == 0 else utb_d).ap(), u_sb[half][:]
                )
                nc.vector.tensor_max(
                    m_sb[half][:], macc[half][:, 0:TS], macc[half][:, TS:]
                )
                nc.sync.dma_start(
                    (ma_d if half == 0 else mb_d).ap(), m_sb[half][:]
                )

        pending = []
        for p in range(NP):
            pt2 = pt_pool.tile([128, 2 * TS], BF16)
            sts = []
            for cc in range(2):
                jj = 2 * p + cc
                st = st_pool.tile([128, TS], F32)
                sts.append(st)
                if jj == 0:
                    pe_touch(ones_sb[:], st)
                    pe_touch(cmt_sb[:], st)
                if jj % NCHUNK == 0 and jj // NCHUNK < 2:
                    # touches only while input DMAs are still in flight
                    k = jj // NCHUNK
                    pe_touch(qt_tiles[k][:], st)
                    pe_touch(qn_sb[:, jj * 128 : jj * 128 + 16], st)
                qk = qt_tiles[jj // NCHUNK]
                off = (jj % NCHUNK) * 128
                for n in range(NN):
                    sl = slice(n * 512, (n + 1) * 512)
                    nc.tensor.matmul(
                        st[:, sl], qk[:, off : off + 128], cmt_sb[:, sl],
                        start=True, stop=True,
                    )
                if not (p % 2 == 1 and cc == 1):
                    nc.scalar.activation(
                        pt2[:, cc * TS : (cc + 1) * TS], st[:],
                        mybir.ActivationFunctionType.Exp,
                        bias=qwb_sb[:, jj : jj + 1],
                    )
            # consume the previous pair BEFORE this pair's Schraudolph op so
            # the Vector FIFO runs MAX(p-1) ahead of tensor_scalar(p)
            if len(pending) > PIPE - 1:
                emit_consume(*pending.pop(0))
            if p % 2 == 1:
                # Schraudolph exp on the Vector engine (u16 bits = bf16)
                jj = 2 * p + 1
                nc.vector.tensor_scalar(
                    pt2[:, TS : 2 * TS].bitcast(mybir.dt.uint16),
                    sts[1][:], A16, qwb2_sb[:, jj : jj + 1],
                    mybir.AluOpType.mult, mybir.AluOpType.add,
                )
            pending.append((p, pt2))
        while pending:
            emit_consume(*pending.pop(0))

        # Z output: copy the z4 bank to SBUF (ScalarE reads PSUM fast),
        # then one partition-strided 8KB DMA of rows 0/32/64/96.
        nc.scalar.copy(z_sb[:], z4_ps[:])
        nc.sync.dma_start(z4_d.ap(), z_sb[0:97:32, :])

    nc.compile()
    return nc


def _host_inputs(c, q, qw, cm):
    import ml_dtypes

    NJ = T // 128
    qT = np.ascontiguousarray(q.T).astype(np.float16)
    qn_re = np.ascontiguousarray(
        q.reshape(NJ, 128, 128).transpose(1, 0, 2).reshape(128, T)
    ).astype(ml_dtypes.bfloat16)
    ones = np.ones((128, 16), dtype=ml_dtypes.bfloat16)
    in_maps = []
    for r in range(NCORES):
        rows = slice(r * TS, (r + 1) * TS)
        cm_r = cm[rows]
        sig2 = (cm_r.astype(np.float64) ** 2).sum(1)
        K = float(qw.max()) + 3.5 * float(np.sqrt(sig2.max()))
        in_maps.append(
            {
                "qT": qT,
                "cmT": np.ascontiguousarray(cm_r.T).astype(np.float16),
                "qn": qn_re,
                "ones": ones,
                "qwb": np.ascontiguousarray(
                    (qw - K).reshape(NJ, 128).T
                ).astype(np.float32),
            }
        )
    return in_maps


def kernel(x, kernel):
    from concourse.bass_utils import run_bass_kernel_spmd

    x = np.asarray(x, dtype=np.float32)
    kern = np.asarray(kernel, dtype=np.float32)
    c, q = x[0, 0], x[1, 0]
    w_c, w_q, w_m = kern[:D], kern[D : 2 * D], kern[2 * D :]

    qw = (q.astype(np.float64) @ w_q.astype(np.float64)).astype(np.float32)
    cm = (c * w_m[None, :]).astype(np.float32)

    if "nc" not in _CACHE:
        _CACHE["nc"] = _build_nc()
    nc = _CACHE["nc"]

    in_maps = _host_inputs(c, q, qw, cm)
    res = run_bass_kernel_spmd(nc, in_maps, list(range(NCORES)))

    U = np.empty((T, D), dtype=np.float64)
    Z = np.empty(T, dtype=np.float64)
    M = np.empty(T, dtype=np.float64)
    for r in range(NCORES):
        rows = slice(r * TS, (r + 1) * TS)
        out = res.results[r]
        U[rows] = (
            np.asarray(out["uta"], dtype=np.float64)
            + np.asarray(out["utb"], dtype=np.float64)
        ).T
        z4 = np.asarray(out["z4"], dtype=np.float64)  # (4, 512)
        zr = np.empty(TS, dtype=np.float64)
        zr[:512] = z4[0] + z4[2]
        zr[512:] = z4[1] + z4[3]
        Z[rows] = zr
        M[rows] = np.maximum(
            np.asarray(out["ma"], dtype=np.float64),
            np.asarray(out["mb"], dtype=np.float64),
        ).max(0)

    U_A = U / Z[:, None]
    b = M / Z
    h = b @ c.astype(np.float64)
    c64 = c.astype(np.float64)
    G = np.concatenate([c64, U_A, c64 * U_A, c64 * h[None, :]], axis=1)
    return G.astype(np.float32)


# revision 18
# speedup vs baseline: 1.1797x; 1.0205x over previous
"""Trainium2 Bass kernel for nn_BiAttentionLayer (T=8192, D=128), 8 NeuronCores.

Math: with context c, question q, kernel w = [w_c | w_q | w_m]:
    S[i,j] = c_i.w_c + q_j.w_q + (c_i*w_m).q_j
    A = softmax_rows(S);  U_A = A @ q
    b = rowmax(A);  h = b @ c;  G = [c, U_A, c*U_A, c*H_A]

Sharding: context rows are split across 8 cores (sequence-parallel over the
rows of the T x T score matrix); q is replicated. Softmax shift is a single
analytic per-core constant K_r (safe fp32 range), removing the row-max pass:
    Pt[j,i] = exp(q_j.(c_i*w_m) + qw_j - K_r)      (j on partitions)

Per core, 64 j-chunks of 128 are processed in pairs (pt2 tiles hold 2
chunks side by side):
    PE : S.T chunk = qT_chunk.T @ cmT            (fp16, fp32 PSUM)
    ACT: pt2 half = exp(S.T + bias)              (bf16 out)
    PE : U.T += qn_chunk.T @ pt half             (2x512-col MMs per chunk)
    PE : Z quads: ones.T @ pt 512-col slices, col-tiled to partitions
         0/32/64/96 of one PSUM bank (concurrent col-group matmuls)
    DVE: one running elementwise max per pair on the full pt2 tile

U and the running max are split into first/second-half accumulators so half
the output DMAs overlap compute. Final row-direction reductions (sum/max
over partitions, b = m/Z, U_A = U.T.T/Z, h, G assembly) run on host.

The output G is (8192, 512) float32.
"""

import sys
from contextlib import ExitStack

import numpy as np

for _p in ("/opt/trn_rl_repo",):
    if _p not in sys.path:
        sys.path.insert(0, _p)

T = 8192
D = 128
NCORES = 8
TS = T // NCORES  # 1024 context rows per core

_CACHE = {}


def _build_nc():
    import concourse.bass as bass
    import concourse.mybir as mybir
    import concourse.tile as tile
    from concourse import bacc

    F32 = mybir.dt.float32
    BF16 = mybir.dt.bfloat16
    F16 = mybir.dt.float16

    NJ = T // 128   # 64 j-chunks
    NP = NJ // 2    # 32 chunk pairs
    NN = TS // 512  # 2 psum column chunks
    PIPE = 2        # consume pair lag
    PT_BUFS = 5     # pt2 pool depth
    OFFLOAD = False  # Schraudolph exp-on-DVE for odd pairs' 2nd chunk
    # Schraudolph exp-on-DVE: u16 = round(A16*s + 128*(127-sigma)) saturates
    # negatives to 0 and its bits ARE bf16(exp(s)) to ~1.8% rms. Offloading
    # some chunks' exp to the Vector engine takes them off the ACT pacer.
    A16 = 128.0 / float(np.log(2.0))
    B16 = 16249.0

    nc = bacc.Bacc("TRN2", target_bir_lowering=False, debug=False)

    qT_d = nc.declare_dram_parameter("qT", [128, T], F16, isOutput=False)
    cmT_d = nc.declare_dram_parameter("cmT", [128, TS], F16, isOutput=False)
    qn_d = nc.declare_dram_parameter("qn", [128, T], BF16, isOutput=False)
    ones_d = nc.declare_dram_parameter("ones", [128, 16], BF16, isOutput=False)
    qwb_d = nc.declare_dram_parameter("qwb", [128, NJ], F32, isOutput=False)

    uta_d = nc.declare_dram_parameter("uta", [128, TS], F32, isOutput=True)
    utb_d = nc.declare_dram_parameter("utb", [128, TS], F32, isOutput=True)
    z4_d = nc.declare_dram_parameter("z4", [4, 512], F32, isOutput=True)
    ma_d = nc.declare_dram_parameter("ma", [128, TS], BF16, isOutput=True)
    mb_d = nc.declare_dram_parameter("mb", [128, TS], BF16, isOutput=True)

    with tile.TileContext(nc) as tc, ExitStack() as ctx:
        const_pool = ctx.enter_context(tc.tile_pool(name="const", bufs=1))
        st_pool = ctx.enter_context(
            tc.tile_pool(name="st", bufs=2, space=bass.MemorySpace.PSUM)
        )
        acc_pool = ctx.enter_context(
            tc.tile_pool(name="acc", bufs=1, space=bass.MemorySpace.PSUM)
        )
        pt_pool = ctx.enter_context(tc.tile_pool(name="pt", bufs=PT_BUFS))

        u_ps = [
            acc_pool.tile([128, 512], F32, tag=f"u{n}", name=f"u{n}")
            for n in range(NN)
        ]
        z4_ps = acc_pool.tile([128, 512], F32, tag="z4", name="z4")

        # ACT table preload: a dummy 1-col exp issued before any data
        # arrives so the ~2.7us ACT_TABLE_LOAD overlaps the input DMAs.
        dummy = const_pool.tile([128, 1], F32, tag="dummy")
        nc.vector.memset(dummy[:], 0.0)
        nc.scalar.activation(
            dummy[:], dummy[:], mybir.ActivationFunctionType.Exp
        )

        # PE warmup spin: full-array matmuls with no DMA deps so the HAM
        # clock-gate reaches K=8/8 while the input DMAs stream in. Results
        # go to u_ps[0], which chunk 0's start=True accumulation clears.
        wm = const_pool.tile([128, 512], BF16, tag="wm")
        nc.vector.memset(wm[:], 0.5)
        for _w in range(4):
            nc.tensor.matmul(
                u_ps[0][:], wm[:, 0:128], wm[:], start=True, stop=True,
                skip_group_check=True,
            )

        NCHUNK = NJ // 8
        cmt_sb = const_pool.tile([128, TS], F16, tag="cmt")
        qt_tiles = [
            const_pool.tile([128, NCHUNK * 128], F16, tag=f"qt{k}", name=f"qt{k}")
            for k in range(8)
        ]
        qn_sb = const_pool.tile([128, NJ * 128], BF16, tag="qn")
        qwb_sb = const_pool.tile([128, NJ], F32, tag="qwb")
        qwb2_sb = const_pool.tile([128, NJ], F32, tag="qwb2")
        act_scr = const_pool.tile([128, 1], F32, tag="act_scr")
        ones_sb = const_pool.tile([128, 16], BF16, tag="ones")
        macc = [
            const_pool.tile([128, 2 * TS], BF16, tag=f"macc{h}", name=f"macc{h}")
            for h in range(2)
        ]
        nc.vector.memset(macc[0][:], 0.0)
        nc.vector.memset(macc[1][:], 0.0)
        u_sb = [
            const_pool.tile([128, TS], F32, tag=f"usb{h}", name=f"usb{h}")
            for h in range(2)
        ]
        m_sb = [
            const_pool.tile([128, TS], BF16, tag=f"msb{h}", name=f"msb{h}")
            for h in range(2)
        ]
        z_sb = const_pool.tile([128, 512], F32, tag="zsb")

        # DMAs, critical-first; per-chunk loads interleaved
        nc.sync.dma_start(cmt_sb[:], cmT_d.ap())
        sl0 = slice(0, NCHUNK * 128)
        nc.sync.dma_start(qt_tiles[0][:], qT_d.ap()[:, sl0])
        nc.sync.dma_start(qwb_sb[:], qwb_d.ap())
        nc.sync.dma_start(ones_sb[:], ones_d.ap())
        # ACT-side touch absorbing the qwb DMA wait
        nc.scalar.copy(act_scr[:], qwb_sb[:, 0:1])
        # per-chunk bias for the DVE-Schraudolph chunks: A16*qwb + B16
        nc.vector.tensor_scalar(
            qwb2_sb[:], qwb_sb[:], A16, B16,
            mybir.AluOpType.mult, mybir.AluOpType.add,
        )
        nc.sync.dma_start(qn_sb[:, sl0], qn_d.ap()[:, sl0])
        for k in range(1, 8):
            sl = slice(k * NCHUNK * 128, (k + 1) * NCHUNK * 128)
            nc.sync.dma_start(qt_tiles[k][:], qT_d.ap()[:, sl])
            nc.sync.dma_start(qn_sb[:, sl], qn_d.ap()[:, sl])

        # PE "touch" matmuls: absorb each DMA's completion wait on the PE so
        # real matmuls carry at most one semaphore wait. Results land in a
        # corner of the current S.T psum tile (overwritten by start=True).
        def pe_touch(ap, st):
            w = min(16, ap.shape[1])
            nc.tensor.matmul(
                st[0:1, 0:w], ap[:, 0:1], ap[:, 0:w],
                start=True, stop=True, skip_group_check=True,
            )

        HALF_P = NP // 2  # u/m half boundary (pairs 0..15 -> half 0)

        def emit_consume(p, pt2):
            half = 0 if p < HALF_P else 1
            # U.T accumulation: 4 x 512-col MMs (2 per chunk)
            for cc in range(2):  # chunk within pair
                jj = 2 * p + cc
                qslice = qn_sb[:, jj * 128 : (jj + 1) * 128]
                for n in range(NN):
                    sl = slice(cc * TS + n * 512, cc * TS + (n + 1) * 512)
                    nc.tensor.matmul(
                        u_ps[n][:], qslice, pt2[:, sl],
                        start=(p % HALF_P) == 0 and cc == 0,
                        stop=(p % HALF_P) == HALF_P - 1 and cc == 1,
                    )
            # Z quads: 4 col-tiled single-row MMs accumulating into
            # partitions 0/32/64/96 of one PSUM bank (concurrent col groups)
            for qd in range(4):
                c = 32 * qd
                nc.tensor.matmul(
                    z4_ps[c : c + 1, :],
                    ones_sb[:, 0:1],
                    pt2[:, qd * 512 : (qd + 1) * 512],
                    start=p == 0, stop=p == NP - 1,
                    skip_group_check=True,
                    tile_position=(0, c),
                )
            # running elementwise max over the whole pair tile
            nc.vector.tensor_max(macc[half][:], macc[half][:], pt2[:])
            if p == HALF_P - 1 or p == NP - 1:
                # drain this half: U psum -> SBUF -> DRAM, fold + ship max.
                # Copies split across Scalar+Vector; the mid-kernel max fold
                # goes to the otherwise-idle GpSimd.
                nc.scalar.copy(u_sb[half][:, 0:512], u_ps[0][:])
                nc.vector.tensor_copy(u_sb[half][:, 512:1024], u_ps[1][:])
                nc.sync.dma_start(
                    (uta_d if half == 0 else utb_d).ap(), u_sb[half][:]
                )
                nc.vector.tensor_max(
                    m_sb[half][:], macc[half][:, 0:TS], macc[half][:, TS:]
                )
                nc.sync.dma_start(
                    (ma_d if half == 0 else mb_d).ap(), m_sb[half][:]
                )

        pending = []
        for p in range(NP):
            pt2 = pt_pool.tile([128, 2 * TS], BF16)
            sts = []
            for cc in range(2):
                jj = 2 * p + cc
                st = st_pool.tile([128, TS], F32)
                sts.append(st)
                if jj == 0:
                    pe_touch(ones_sb[:], st)
                    pe_touch(cmt_sb[:], st)
                if jj % NCHUNK == 0 and jj // NCHUNK < 2:
                    # touches only while input DMAs are still in flight
                    k = jj // NCHUNK
                    pe_touch(qt_tiles[k][:], st)
                    pe_touch(qn_sb[:, jj * 128 : jj * 128 + 16], st)
                qk = qt_tiles[jj // NCHUNK]
                off = (jj % NCHUNK) * 128
                for n in range(NN):
                    sl = slice(n * 512, (n + 1) * 512)
                    nc.tensor.matmul(
                        st[:, sl], qk[:, off : off + 128], cmt_sb[:, sl],
                        start=True, stop=True,
                    )
                if not (OFFLOAD and p % 2 == 1 and cc == 1):
                    nc.scalar.activation(
                        pt2[:, cc * TS : (cc + 1) * TS], st[:],
                        mybir.ActivationFunctionType.Exp,
                        bias=qwb_sb[:, jj : jj + 1],
                    )
            # consume the previous pair BEFORE this pair's Schraudolph op so
            # the Vector FIFO runs MAX(p-1) ahead of tensor_scalar(p)
            if len(pending) > PIPE - 1:
                emit_consume(*pending.pop(0))
            if OFFLOAD and p % 2 == 1:
                # Schraudolph exp on the Vector engine (u16 bits = bf16)
                jj = 2 * p + 1
                nc.vector.tensor_scalar(
                    pt2[:, TS : 2 * TS].bitcast(mybir.dt.uint16),
                    sts[1][:], A16, qwb2_sb[:, jj : jj + 1],
                    mybir.AluOpType.mult, mybir.AluOpType.add,
                )
            pending.append((p, pt2))
        while pending:
            emit_consume(*pending.pop(0))

        # Z output: copy the z4 bank to SBUF (ScalarE reads PSUM fast),
        # then one partition-strided 8KB DMA of rows 0/32/64/96.
        nc.scalar.copy(z_sb[:], z4_ps[:])
        nc.sync.dma_start(z4_d.ap(), z_sb[0:97:32, :])

    nc.compile()
    return nc


def _host_inputs(c, q, qw, cm):
    import ml_dtypes

    NJ = T // 128
    qT = np.ascontiguousarray(q.T).astype(np.float16)
    qn_re = np.ascontiguousarray(
        q.reshape(NJ, 128, 128).transpose(1, 0, 2).reshape(128, T)
    ).astype(ml_dtypes.bfloat16)
    ones = np.ones((128, 16), dtype=ml_dtypes.bfloat16)
    in_maps = []
    for r in range(NCORES):
        rows = slice(r * TS, (r + 1) * TS)
        cm_r = cm[rows]
        sig2 = (cm_r.astype(np.float64) ** 2).sum(1)
        K = float(qw.max()) + 3.5 * float(np.sqrt(sig2.max()))
        in_maps.append(
            {
                "qT": qT,
                "cmT": np.ascontiguousarray(cm_r.T).astype(np.float16),
                "qn": qn_re,
                "ones": ones,
                "qwb": np.ascontiguousarray(
                    (qw - K).reshape(NJ, 128).T
                ).astype(np.float32),
            }
        )
    return in_maps


def kernel(x, kernel):
    from concourse.bass_utils import run_bass_kernel_spmd

    x = np.asarray(x, dtype=np.float32)
    kern = np.asarray(kernel, dtype=np.float32)
    c, q = x[0, 0], x[1, 0]
    w_c, w_q, w_m = kern[:D], kern[D : 2 * D], kern[2 * D :]

    qw = (q.astype(np.float64) @ w_q.astype(np.float64)).astype(np.float32)
    cm = (c * w_m[None, :]).astype(np.float32)

    if "nc" not in _CACHE:
        _CACHE["nc"] = _build_nc()
    nc = _CACHE["nc"]

    in_maps = _host_inputs(c, q, qw, cm)
    res = run_bass_kernel_spmd(nc, in_maps, list(range(NCORES)))

    U = np.empty((T, D), dtype=np.float64)
    Z = np.empty(T, dtype=np.float64)
    M = np.empty(T, dtype=np.float64)
    for r in range(NCORES):
        rows = slice(r * TS, (r + 1) * TS)
        out = res.results[r]
        U[rows] = (
            np.asarray(out["uta"], dtype=np.float64)
            + np.asarray(out["utb"], dtype=np.float64)
        ).T
        z4 = np.asarray(out["z4"], dtype=np.float64)  # (4, 512)
        zr = np.empty(TS, dtype=np.float64)
        zr[:512] = z4[0] + z4[2]
        zr[512:] = z4[1] + z4[3]
        Z[rows] = zr
        M[rows] = np.maximum(
            np.asarray(out["ma"], dtype=np.float64),
            np.asarray(out["mb"], dtype=np.float64),
        ).max(0)

    U_A = U / Z[:, None]
    b = M / Z
    h = b @ c.astype(np.float64)
    c64 = c.astype(np.float64)
    G = np.concatenate([c64, U_A, c64 * U_A, c64 * h[None, :]], axis=1)
    return G.astype(np.float32)
